# revision 5
# baseline (speedup 1.0000x reference)
"""Bidirectional LSTM Trainium2 kernel.

Strategy: one NeuronCore per direction (core 0 fwd, core 1 bwd on time-reversed
inputs). Each core runs three phases:
  X: input projection xg = x @ W_ih^T (+bias later), quarter-permuted gate cols,
     stored bf16 in DRAM as [T*64, 2048] (t-major rows).
  R: the serial recurrence, 64 x For_i iterations of 8 unrolled steps.
     Vertical-packed layout: PSUM bank b holds quarters (2b, 2b+1) stacked on
     partitions (batch 0-63 / 64-127); xg + bias enter the PSUM via an
     identity-matmul accumulation; gate nonlinearities on ACT; c/h chain on DVE;
     h transposed back to [H, B] via one PE transpose per bank.
  F: trailing linear partial out^T = W1 @ h_seq (+b_emb on core 0 only),
     written as [512, T*64]; host sums the two cores' partials.
All matmul operands bf16 (fp32 PSUM accumulate); c state fp32.
"""
import sys, os
sys.path.insert(0, '/opt/trn_rl_repo')
import numpy as np
import ml_dtypes

import concourse.bass as bass
import concourse.mybir as mybir
import concourse.tile as tile
from concourse import bacc
from concourse import bass_utils
from concourse.bass import ds
from concourse.bass_interp import get_hw_module

F32 = mybir.dt.float32
BF16 = mybir.dt.bfloat16
AF = mybir.ActivationFunctionType
OP = mybir.AluOpType

B, H, NIN, NOUT = 64, 512, 512, 512
NG = 4 * H  # 2048
KT = 4

_BUILD_CACHE = {}


def _build(T):
    if T in _BUILD_CACHE:
        return _BUILD_CACHE[T]
    R = T * B  # total rows
    nc = bacc.Bacc("TRN2", target_bir_lowering=False, debug=False,
                   enable_asserts=True, num_devices=2)
    xT_d = nc.dram_tensor("xT", (NIN, R), BF16, kind="ExternalInput").ap()
    wih_d = nc.dram_tensor("wih", (NIN, NG), BF16, kind="ExternalInput").ap()
    whh_d = nc.dram_tensor("whh", (H, NG), BF16, kind="ExternalInput").ap()
    brow_d = nc.dram_tensor("brow", (1, NG), BF16, kind="ExternalInput").ap()
    ib_d = nc.dram_tensor("ib", (128, 64), BF16, kind="ExternalInput").ap()
    idn_d = nc.dram_tensor("idn", (128, 128), BF16, kind="ExternalInput").ap()
    w1t_d = nc.dram_tensor("w1t", (H, NOUT), BF16, kind="ExternalInput").ap()
    bemb_d = nc.dram_tensor("bemb", (128, 4), F32, kind="ExternalInput").ap()
    xg_d = nc.dram_tensor("xgd", (R, NG), BF16, kind="Internal").ap()
    hsq_d = nc.dram_tensor("hsqd", (4, 128, R), BF16, kind="Internal").ap()
    out_d = nc.dram_tensor("outT", (NOUT, R), F32, kind="ExternalOutput").ap()

    with tile.TileContext(nc) as tc:
        with tc.tile_pool(name="wpool", bufs=1) as wp, \
             tc.tile_pool(name="mpool", bufs=1) as mp:
            # persistent weights
            wih = []
            whh = []
            for k in range(KT):
                t = wp.tile([128, NG], BF16, tag=f"wih{k}", name=f"wih{k}")
                nc.sync.dma_start(out=t, in_=wih_d[k*128:(k+1)*128, :])
                wih.append(t)
                t2 = wp.tile([128, NG], BF16, tag=f"whh{k}", name=f"whh{k}")
                nc.sync.dma_start(out=t2, in_=whh_d[k*128:(k+1)*128, :])
                whh.append(t2)
            w1t = []
            for k in range(KT):
                t = wp.tile([128, NOUT], BF16, tag=f"w1t{k}", name=f"w1t{k}")
                nc.sync.dma_start(out=t, in_=w1t_d[k*128:(k+1)*128, :])
                w1t.append(t)
            ib = mp.tile([128, 64], BF16, tag="ib")
            nc.sync.dma_start(out=ib, in_=ib_d)
            idn = mp.tile([128, 128], BF16, tag="idn")
            nc.sync.dma_start(out=idn, in_=idn_d)
            bemb = mp.tile([128, 4], F32, tag="bemb")
            nc.sync.dma_start(out=bemb, in_=bemb_d)

            # ------- Phases X+R interleaved: X fills PE bubbles in R -------
            # Lookahead LA=32 steps: prologue computes xg rows [0, 2048);
            # each main-loop iteration runs 16 R steps and 8 X M-tiles for
            # rows one LA ahead. For_i back-edge barriers order X->R DRAM RAW.
            with tc.tile_pool(name="rs", bufs=1) as rs, \
                 tc.tile_pool(name="rps", bufs=2, space="PSUM") as rpp:

                def emit_xtile_mms(row, tag_i, nm):
                    xk = []
                    for k in range(KT):
                        t = rs.tile([128, 128], BF16, tag=f"xk{k}", bufs=4,
                                    name=f"xk{nm}_{k}")
                        nc.sync.dma_start(out=t, in_=xT_d[k*128:(k+1)*128, row])
                        xk.append(t)
                    pss = []
                    for c in range(4):
                        ps = rpp.tile([128, 512], F32, tag=f"xps{(tag_i + c) % 2}",
                                      bufs=1, name=f"xps{nm}_{c}")
                        for k in range(KT):
                            nc.tensor.matmul(ps, xk[k], wih[k][:, c*512:(c+1)*512],
                                             start=(k == 0), stop=(k == KT-1))
                        pss.append(ps)
                    return pss

                def emit_xtile_copies(pss, row, nm):
                    for c in range(4):
                        sb = rs.tile([128, 512], BF16, tag=f"xsb{c%2}", bufs=4,
                                     name=f"xsb{nm}_{c}")
                        if c % 2 == 0:
                            nc.vector.tensor_copy(sb, pss[c])
                        else:
                            nc.scalar.activation(sb, pss[c], AF.Copy)
                        nc.sync.dma_start(out=xg_d[row, c*512:(c+1)*512], in_=sb)

                # prologue: xg for the first LA steps (plus handle small T)
                LA = 32
                interleave = T >= 3 * LA // 2 and (T - LA) % 16 == 0
                n_pro = (LA * B // 128) if interleave else (R // 128)
                for mt in range(n_pro):
                    pss = emit_xtile_mms(slice(mt*128, (mt+1)*128), mt, f"p{mt}")
                    emit_xtile_copies(pss, slice(mt*128, (mt+1)*128), f"p{mt}")

                hTp = [mp.tile([128, 128], BF16, tag=f"hTp{b}", name=f"hTp{b}")
                       for b in range(2)]
                cst = [mp.tile([128, 128], F32, tag=f"cst{b}", name=f"cst{b}")
                       for b in range(2)]
                for t in hTp:
                    nc.vector.memset(t, 0.0)
                for t in cst:
                    nc.vector.memset(t, 0.0)
                NXG = 4
                xgt = [mp.tile([128, NG], BF16, tag=f"xgt{j}", name=f"xgt{j}")
                       for j in range(NXG)]
                for j in range(NXG):
                    nc.vector.memset(xgt[j][64:128, :], 0.0)
                    nc.sync.dma_start(out=xgt[j][64:65, :], in_=brow_d)

                fk_cache = {}

                def emit_fchunk(rowslc, nm):
                    rk = []
                    for k in range(KT):
                        t = rs.tile([128, 512], BF16, tag=f"fk{k}", bufs=2,
                                    name=f"fk{nm}_{k}")
                        nc.sync.dma_start(out=t, in_=hsq_d[k][:, rowslc])
                        rk.append(t)
                    for m in range(4):
                        ps = rpp.tile([128, 512], F32, tag=f"xps{m%2}", bufs=1,
                                      name=f"fps{nm}_{m}")
                        for k in range(KT):
                            nc.tensor.matmul(ps, w1t[k][:, m*128:(m+1)*128], rk[k],
                                             start=(k == 0), stop=(k == KT-1))
                        ob = rs.tile([128, 512], F32, tag=f"ob{m%2}", bufs=4,
                                     name=f"ob{nm}_{m}")
                        if m % 2 == 0:
                            nc.scalar.activation(ob, ps, AF.Identity,
                                                 bias=bemb[:, m:m+1])
                        else:
                            nc.vector.tensor_scalar_add(ob, ps, bemb[:, m:m+1])
                        nc.sync.dma_start(out=out_d[m*128:(m+1)*128, rowslc],
                                          in_=ob)

                UNROLL = 16

                def emit_step_c(s, r0, with_x):
                    emit_step(s, r0, with_x)

                def emit_step(s, r0, with_x):
                    xt = xgt[s % NXG]
                    nc.sync.dma_start(out=xt[0:64, :],
                                      in_=xg_d[ds(r0 + s*64, 64), :])
                    pss = []
                    for b in range(2):
                        ps = rpp.tile([128, 512], F32, tag=f"g{b}", bufs=2,
                                      name=f"ps{s}_{b}")
                        q0, q1 = 2*b, 2*b + 1
                        nc.tensor.matmul(ps[0:64, :], ib, xt[:, q0*512:(q0+1)*512],
                                         start=True, stop=False,
                                         tile_position=(0, 0), skip_group_check=True)
                        nc.tensor.matmul(ps[64:128, :], ib, xt[:, q1*512:(q1+1)*512],
                                         start=True, stop=False,
                                         tile_position=(0, 64), skip_group_check=True)
                        for k in range(KT):
                            last = (k == KT - 1)
                            hTk = hTp[k // 2][:, (k % 2)*64:(k % 2 + 1)*64]
                            nc.tensor.matmul(ps[0:64, :], hTk,
                                             whh[k][:, q0*512:(q0+1)*512],
                                             start=False, stop=last,
                                             tile_position=(0, 0),
                                             skip_group_check=True)
                            nc.tensor.matmul(ps[64:128, :], hTk,
                                             whh[k][:, q1*512:(q1+1)*512],
                                             start=False, stop=last,
                                             tile_position=(0, 64),
                                             skip_group_check=True)
                        pss.append(ps)
                    xps = None
                    if with_x and s % 2 == 1:
                        xrow = ds(r0 + LA*64 + ((s-1)//2)*128, 128)
                        xps = emit_xtile_mms(xrow, (s-1)//2, f"x{s}")
                    for b in range(2):
                        ps = pss[b]
                        tg = rs.tile([128, 128], F32, tag=f"tg{b}", bufs=2,
                                     name=f"tg{s}_{b}")
                        nc.scalar.activation(tg, ps[:, 384:512], AF.Tanh)
                        sg = rs.tile([128, 384], F32, tag=f"sg{b}", bufs=2,
                                     name=f"sg{s}_{b}")
                        nc.scalar.activation(sg, ps[:, 0:384], AF.Sigmoid)
                        u = rs.tile([128, 128], F32, tag=f"u{b}", bufs=2,
                                    name=f"u{s}_{b}")
                        nc.vector.tensor_tensor(u, sg[:, 0:128], tg, OP.mult)
                        t1 = rs.tile([128, 128], F32, tag=f"t1{b}", bufs=2,
                                     name=f"t1{s}_{b}")
                        nc.vector.tensor_tensor(t1, sg[:, 128:256], cst[b], OP.mult)
                        nc.vector.tensor_tensor(cst[b], u, t1, OP.add)
                        tct = rs.tile([128, 128], F32, tag=f"tc{b}", bufs=2,
                                      name=f"tc{s}_{b}")
                        nc.scalar.activation(tct, cst[b], AF.Tanh)
                        hp = rs.tile([128, 128], BF16, tag=f"hp{b}", bufs=2,
                                     name=f"hp{s}_{b}")
                        nc.vector.tensor_tensor(hp, sg[:, 256:384], tct, OP.mult)
                        psT = rpp.tile([128, 128], BF16, tag=f"pt{b}", bufs=1,
                                       name=f"psT{s}_{b}")
                        nc.tensor.transpose(psT, hp, idn)
                        nc.vector.tensor_copy(hTp[b], psT)
                        nc.sync.dma_start(out=hsq_d[2*b][:, ds(r0 + s*64, 64)],
                                          in_=hTp[b][:, 0:64])
                        nc.sync.dma_start(out=hsq_d[2*b+1][:, ds(r0 + s*64, 64)],
                                          in_=hTp[b][:, 64:128])
                    if xps is not None:
                        xrow = ds(r0 + LA*64 + ((s-1)//2)*128, 128)
                        emit_xtile_copies(xps, xrow, f"x{s}")

                if interleave:
                    IT = UNROLL * 64  # 1024 rows per iteration
                    for s in range(UNROLL):  # first iteration straight-line
                        emit_step_c(s, 0, with_x=True)
                    with tc.For_i(IT, (T - LA) * B, IT) as r0:
                        for s in range(UNROLL):
                            emit_step(s, r0, with_x=True)
                            if s == 3:
                                emit_fchunk(ds(r0 - IT, 512), "fa")
                            elif s == 11:
                                emit_fchunk(ds(r0 - IT + 512, 512), "fb")
                    with tc.For_i((T - LA) * B, R, IT) as r0:
                        for s in range(UNROLL):
                            emit_step(s, r0, with_x=False)
                    # epilogue F: rows not covered in-loop
                    f_done = (T - LA) * B - IT
                    for rc in range(f_done // 512, R // 512):
                        emit_fchunk(slice(rc*512, (rc+1)*512), f"e{rc}")
                else:
                    with tc.For_i(0, R, UNROLL * 64) as r0:
                        for s in range(UNROLL):
                            emit_step(s, r0, with_x=False)
                    for rc in range(R // 512):
                        emit_fchunk(slice(rc*512, (rc+1)*512), f"e{rc}")

    nc.compile()
    _BUILD_CACHE[T] = nc
    return nc


def _gate_perm():
    # chunk q (512 cols) = [i_q | f_q | o_q | g~_q], each 128 wide
    perm = np.zeros(NG, np.int64)
    for q in range(4):
        base = q * 512
        perm[base + 0:base + 128] = 0 * 512 + q * 128 + np.arange(128)    # i
        perm[base + 128:base + 256] = 1 * 512 + q * 128 + np.arange(128)  # f
        perm[base + 256:base + 384] = 3 * 512 + q * 128 + np.arange(128)  # o
        perm[base + 384:base + 512] = 2 * 512 + q * 128 + np.arange(128)  # g~
    return perm


def _host_inputs(T, inputs, w_ih, w_hh, b_ih, b_hh, w1, bemb_vec, reverse):
    bf = ml_dtypes.bfloat16
    perm = _gate_perm()
    x = inputs  # [B, T, NIN]
    if reverse:
        x = x[:, ::-1, :]
    xT = np.ascontiguousarray(x.transpose(2, 1, 0).reshape(NIN, T * B)).astype(bf)
    wihp = np.ascontiguousarray(w_ih.T[:, perm]).astype(bf)
    whhp = np.ascontiguousarray(w_hh.T[:, perm]).astype(bf)
    brow = (b_ih + b_hh)[perm].reshape(1, NG).astype(bf)
    ibm = np.zeros((128, 64), np.float32)
    ibm[0:64, 0:64] = np.eye(64)
    ibm[64, :] = 1.0
    idn = np.eye(128, dtype=np.float32)
    w1t = np.ascontiguousarray(w1.T).astype(bf)  # [H, NOUT]
    bemb_t = np.zeros((128, 4), np.float32)
    for m in range(4):
        bemb_t[:, m] = bemb_vec[m*128:(m+1)*128]
    return {
        "xT": xT, "wih": wihp, "whh": whhp, "brow": brow,
        "ib": ibm.astype(bf), "idn": idn.astype(bf), "w1t": w1t,
        "bemb": bemb_t,
    }


def kernel(inputs, w_ih_f, w_hh_f, b_ih_f, b_hh_f,
           w_ih_b, w_hh_b, b_ih_b, b_hh_b, w_emb, b_emb):
    inputs = np.asarray(inputs, np.float32)
    T = inputs.shape[1]
    nc = _build(T)
    in0 = _host_inputs(T, inputs, np.asarray(w_ih_f, np.float32),
                       np.asarray(w_hh_f, np.float32),
                       np.asarray(b_ih_f, np.float32),
                       np.asarray(b_hh_f, np.float32),
                       np.asarray(w_emb, np.float32)[:, 0:H],
                       np.asarray(b_emb, np.float32), reverse=False)
    in1 = _host_inputs(T, inputs, np.asarray(w_ih_b, np.float32),
                       np.asarray(w_hh_b, np.float32),
                       np.asarray(b_ih_b, np.float32),
                       np.asarray(b_hh_b, np.float32),
                       np.asarray(w_emb, np.float32)[:, H:2*H],
                       np.zeros(NOUT, np.float32), reverse=True)
    hw_m = get_hw_module(nc.m)
    old_m = nc.m
    nc.m = hw_m
    try:
        res = bass_utils.run_bass_kernel_spmd(nc, [in0, in1], core_ids=[0, 1])
    finally:
        nc.m = old_m
    out0 = res.results[0]["outT"].reshape(NOUT, T, B)
    out1 = res.results[1]["outT"].reshape(NOUT, T, B)[:, ::-1, :]
    out = (out0 + out1).transpose(2, 1, 0)
    return np.ascontiguousarray(out).astype(np.float32)
